# revision 30
# baseline (speedup 1.0000x reference)
"""Trainium2 Bass kernel: per-point 3x3 Gaussian covariance from quaternion + log_scale.

cov = R diag(exp(log_scale)) R^T with R built from the normalized quaternion.

v4: fused-instruction design.
- Host uploads PLANAR fp16 component blocks concatenated per partition row:
  q_cat [P, 4, R] in component order (w, z, y, x), l_cat [P, 3, R].
  Host reassembles the 6 unique cov entries (out_cat [P, 6, R] fp16) into the
  [N,3,3] fp32 output.
- All intermediates fp16 -> DVE tensor_tensor runs in 2x mode.
- Concatenated work tiles + strided/broadcast access patterns fuse groups of
  identical elementwise ops into single wide DVE instructions (e.g. all 3
  columns of M = H diag(sh) in one op), cutting per-instruction fixed cost and
  semaphore traffic.
- ScalarE does the squares (of q and of M) and exps in 4 activations per tile,
  all from one activation table set; 1/(n2/2) via custom-DVE
  reciprocal_approx_fast (fp32).

Math: with half-squares hc = c^2/2 and H = (n2/2) R:
  pa = hw - hz, pb = hx - hy, pc = hw + hz, pd = hx + hy
  h00 = pa + pb, h11 = pa - pb, h22 = pc - pd, n2h = pc + pd
  h01 = xy - wz, h10 = xy + wz, h02 = xz + wy, h20 = xz - wy,
  h12 = yz - wx, h21 = yz + wx
  sh_j = exp(ls_j/2) / n2h;  M = H diag(sh);  cov = M M^T.
"""

import os
import numpy as np

import concourse.bass as bass
import concourse.bacc as bacc
import concourse.mybir as mybir
from concourse.tile import TileContext
from concourse.bass_utils import run_bass_kernel_spmd

AF = mybir.ActivationFunctionType
FP32 = mybir.dt.float32
FP16 = mybir.dt.float16

N_CORES = 8
N_FULL = 4_000_000
P = 128
R = 3920                      # rows per partition per core; 128*3920*8 = 4,014,080 >= N
NPC = P * R                   # points per core (padded)
F = int(os.environ.get("KERNEL_F", "980"))       # points per partition per tile
NT = R // F                                      # tiles per core (uniform)
assert NT * F == R
WORK_BUFS = int(os.environ.get("WORK_BUFS", "1"))
IO_BUFS = int(os.environ.get("IO_BUFS", "2"))
NO_BCAST = os.environ.get("NO_BCAST", "0") == "1"   # fallback: no stride-0 operands
SPLIT_OUT = os.environ.get("SPLIT_OUT", "1") == "1"  # two out-DMAs per tile

SQRT_HALF = 0.7071067811865476

# q_cat component order
QW, QZ, QY, QX = 0, 1, 2, 3

_built = {}


def _build():
    key = (F, WORK_BUFS, IO_BUFS, NO_BCAST, SPLIT_OUT)
    if key in _built:
        return _built[key]

    nc = bacc.Bacc("TRN2", target_bir_lowering=False, debug=False, num_devices=N_CORES)
    # tile-major layouts: each tile's data is contiguous per partition
    q_cat = nc.dram_tensor("q_cat", [P, NT, 4 * F], FP16, kind="ExternalInput")
    l_cat = nc.dram_tensor("l_cat", [P, NT, 3 * F], FP16, kind="ExternalInput")
    o_cat = nc.dram_tensor("o_cat", [P, NT, 6 * F], FP16, kind="ExternalOutput")

    qv, lv, ov = q_cat.ap(), l_cat.ap(), o_cat.ap()

    with TileContext(nc) as tc:
        with (
            tc.tile_pool(name="io", bufs=IO_BUFS) as io,
            tc.tile_pool(name="otp", bufs=IO_BUFS) as ot_pool,
            tc.tile_pool(name="wk", bufs=WORK_BUFS) as wk,
        ):
            for ti in range(NT):
                _tile_body(nc, io, ot_pool, wk, qv, lv, ov, ti, F)

    nc.compile()
    _built[key] = nc
    return nc


def _tile_body(nc, io, ot_pool, wk, qv, lv, ov, ti, f):
    t0 = ti * f

    def wt(tag, units, dt=FP16):
        return wk.tile([P, units * f], dt, tag=tag, name=f"{tag}_{t0}")

    V = nc.vector

    # ---- DMA in (contiguous per partition) -------------------------------
    qt = io.tile([P, 4 * f], FP16, tag="q", name=f"q_{t0}")
    lt = io.tile([P, 3 * f], FP16, tag="l", name=f"l_{t0}")
    nc.sync.dma_start(out=qt, in_=qv[:, ti, :])
    nc.sync.dma_start(out=lt, in_=lv[:, ti, :])
    q3 = qt.rearrange("p (k f) -> p k f", k=4)

    # ---- ScalarE: half-squares (one act), exps (one act) -----------------
    hsq = wt("hsq", 4)
    nc.scalar.activation(hsq, qt, AF.Square, scale=SQRT_HALF)
    hs3 = hsq.rearrange("p (k f) -> p k f", k=4)   # (hw, hz, hy, hx)
    ec = wt("ec", 3)
    nc.scalar.activation(ec, lt, AF.Exp, scale=0.5)

    # ---- pair products first: DVE needs only the q DMA -------------------
    # p layout [pxy pxz pzy pwz pwy pwx]
    pp = wt("pp", 6)
    p6 = pp.rearrange("p (k f) -> p k f", k=6)
    if NO_BCAST:
        for j, qi in enumerate((QZ, QY, QX)):
            V.tensor_mul(p6[:, 3 + j, :], q3[:, QW, :], q3[:, qi, :])
    else:
        wb = q3[:, QW:QW + 1, :].broadcast_to((P, 3, f))
        V.tensor_mul(p6[:, 3:6, :], wb, q3[:, 1:4, :])   # (pwz, pwy, pwx)
    V.tensor_mul(p6[:, 0, :], q3[:, QX, :], q3[:, QY, :])  # pxy
    V.tensor_mul(p6[:, 1, :], q3[:, QX, :], q3[:, QZ, :])  # pxz
    V.tensor_mul(p6[:, 2, :], q3[:, QZ, :], q3[:, QY, :])  # pzy = yz

    # ---- diagonal butterfly ---------------------------------------------
    # spd layout: [pa pb pc pd]
    spd = wt("spd", 4)
    sp3 = spd.rearrange("p (k f) -> p k f", k=4)
    in0 = hs3[:, 0:4:3, :]                # (hw, hx)
    in1 = hs3[:, 1:3, :]                  # (hz, hy)
    V.tensor_sub(sp3[:, 0:2, :], in0, in1)     # (pa, pb)
    V.tensor_add(sp3[:, 2:4, :], in0, in1)     # (pc, pd)
    # hdiag layout: [h00 h11 h22 n2h]
    hd = wt("hd", 4)
    hd3 = hd.rearrange("p (k f) -> p k f", k=4)
    i0 = sp3[:, 0:4:2, :]                 # (pa, pc)
    i1 = sp3[:, 1:4:2, :]                 # (pb, pd)
    V.tensor_sub(hd3[:, 1:3, :], i0, i1)       # (h11, h22)
    V.tensor_add(hd3[:, 0:4:3, :], i0, i1)     # (h00, n2h)

    # ---- reciprocal of n2h ----------------------------------------------
    n2h32 = wt("n2h32", 1, FP32)
    nc.scalar.copy(out=n2h32, in_=hd[:, 3 * f:4 * f])
    inv32 = wt("inv32", 1, FP32)
    V.reciprocal_approx_fast(out=inv32, in_=n2h32)
    inv = wt("inv", 1)
    nc.scalar.copy(out=inv, in_=inv32)

    # ---- off-diagonal H (overlaps the ScalarE inv copy) -----------------
    # ho layout [h01 h20 h12 | h10 h02 h21]
    ho = wt("ho", 6)
    V.tensor_sub(ho[:, 0:3 * f], pp[:, 0:3 * f], pp[:, 3 * f:6 * f])
    V.tensor_add(ho[:, 3 * f:6 * f], pp[:, 0:3 * f], pp[:, 3 * f:6 * f])

    # ---- sh = e * inv (broadcast) ---------------------------------------
    sh = wt("sh", 3)
    if NO_BCAST:
        for j in range(3):
            V.tensor_mul(sh[:, j * f:(j + 1) * f], ec[:, j * f:(j + 1) * f], inv)
    else:
        invb = inv.rearrange("p (k f) -> p k f", k=1).broadcast_to((P, 3, f))
        V.tensor_mul(sh.rearrange("p (k f) -> p k f", k=3),
                     ec.rearrange("p (k f) -> p k f", k=3), invb)

    # ---- M = H diag(sh); M layout j-major: [M00 M10 M20 M01 M11 M21 M02 M12 M22]
    mm = wt("mm", 9)
    m9 = mm.rearrange("p (k f) -> p k f", k=9)
    # diagonal: (h00,h11,h22) * (sh0,sh1,sh2) -> slots 0,4,8
    sh3 = sh.rearrange("p (k f) -> p k f", k=3)
    V.tensor_mul(m9[:, 0:9:4, :], hd3[:, 0:3, :], sh3)
    # off-diagonals, pair-fused: ho=[h01 h20 h12 h10 h02 h21], slot = 3j+i
    ho6 = ho.rearrange("p (k f) -> p k f", k=6)
    # (h01,h12)*(sh1,sh2) -> M(3,7); (h20,h21)*(sh0,sh1) -> M(2,5);
    # (h10,h02)*(sh0,sh2) -> M(1,6)
    V.tensor_mul(m9[:, 3:8:4, :], ho6[:, 0:3:2, :], sh3[:, 1:3, :])
    V.tensor_mul(m9[:, 2:6:3, :], ho6[:, 1:6:4, :], sh3[:, 0:2, :])
    V.tensor_mul(m9[:, 1:7:5, :], ho6[:, 3:5, :], sh3[:, 0:3:2, :])

    # ---- ScalarE: squares of M (one act) --------------------------------
    msq = wt("msq", 9)
    nc.scalar.activation(msq, mm, AF.Square)
    mq9 = msq.rearrange("p (k f) -> p k f", k=9)

    # ---- gram ------------------------------------------------------------
    ot = ot_pool.tile([P, 6 * f], FP16, tag="oc", name=f"oc_{t0}")
    o6 = ot.rearrange("p (k f) -> p k f", k=6)
    # off-diag products: g layout pair-major [(01)j0 j1 j2 | (02)... | (12)...]
    gg = wt("gg", 9)
    g9 = gg.rearrange("p (k f) -> p k f", k=9)
    if NO_BCAST:
        for pi, (i, k) in enumerate(((0, 1), (0, 2), (1, 2))):
            V.tensor_mul(g9[:, 3 * pi:3 * (pi + 1), :],
                         m9[:, i:9:3, :], m9[:, k:9:3, :])
    else:
        # pairs (0,1) and (0,2) share row0: one 6f op with a broadcast row0
        mij = mm.rearrange("p (j i f) -> p i j f", j=3, i=3)
        row0b = mij[:, 0:1, :, :].broadcast_to((P, 2, 3, f))
        g01_02 = gg[:, 0:6 * f].rearrange("p (q j f) -> p q j f", q=2, j=3)
        V.tensor_mul(g01_02, row0b, mij[:, 1:3, :, :])
        V.tensor_mul(g9[:, 6:9, :], m9[:, 1:9:3, :], m9[:, 2:9:3, :])
    s3 = wt("s3", 3)
    V.tensor_add(s3.rearrange("p (k f) -> p k f", k=3),
                 g9[:, 0:9:3, :], g9[:, 1:9:3, :])
    V.tensor_add(o6[:, 0:3, :],
                 s3.rearrange("p (k f) -> p k f", k=3), g9[:, 2:9:3, :])
    if SPLIT_OUT:
        nc.sync.dma_start(out=ov[:, ti, 0:3 * f], in_=ot[:, 0:3 * f])
    # diag: cov_ii = sum_j msq[3j+i]
    sd = wt("sd", 3)
    V.tensor_add(sd, msq[:, 0:3 * f], msq[:, 3 * f:6 * f])
    V.tensor_add(o6[:, 3:6, :],
                 sd.rearrange("p (k f) -> p k f", k=3), mq9[:, 6:9, :])
    if SPLIT_OUT:
        nc.sync.dma_start(out=ov[:, ti, 3 * f:6 * f], in_=ot[:, 3 * f:6 * f])
    else:
        nc.sync.dma_start(out=ov[:, ti, :], in_=ot)


def _pad_and_shard(quaternion, log_scale):
    n = quaternion.shape[0]
    ntot = N_CORES * NPC
    q16 = np.empty((ntot, 4), np.float16)
    l16 = np.empty((ntot, 3), np.float16)
    q16[:n] = quaternion.astype(np.float16)
    l16[:n] = log_scale.astype(np.float16)
    q16[n:] = np.array([1, 0, 0, 0], np.float16)
    l16[n:] = 0
    in_maps = []
    comp_order = (0, 3, 2, 1)   # (w, z, y, x) columns of quaternion
    for i in range(N_CORES):
        sl = slice(i * NPC, (i + 1) * NPC)
        qc = np.empty((P, NT, 4, F), np.float16)
        for k, ci in enumerate(comp_order):
            qc[:, :, k, :] = q16[sl, ci].reshape(P, NT, F)
        lc = np.empty((P, NT, 3, F), np.float16)
        for ci in range(3):
            lc[:, :, ci, :] = l16[sl, ci].reshape(P, NT, F)
        in_maps.append({"q_cat": qc.reshape(P, NT, 4 * F),
                        "l_cat": lc.reshape(P, NT, 3 * F)})
    return in_maps


def kernel_with_stats(quaternion, log_scale, trace=False):
    quaternion = np.asarray(quaternion, dtype=np.float32)
    log_scale = np.asarray(log_scale, dtype=np.float32)
    n = quaternion.shape[0]
    nc = _build()
    in_maps = _pad_and_shard(quaternion, log_scale)
    res = run_bass_kernel_spmd(nc, in_maps, core_ids=list(range(N_CORES)), trace=trace)
    out = np.empty((n, 3, 3), np.float32)
    # o_cat slots: [c01 c02 c12 | c00 c11 c22]
    slots = {0: [(0, 1), (1, 0)], 1: [(0, 2), (2, 0)], 2: [(1, 2), (2, 1)],
             3: [(0, 0)], 4: [(1, 1)], 5: [(2, 2)]}
    for s, ps in slots.items():
        full = np.concatenate(
            [r["o_cat"].reshape(P, NT, 6, F)[:, :, s, :].reshape(-1)
             for r in res.results])[:n]
        full = full.astype(np.float32)
        for (i, k) in ps:
            out[:, i, k] = full
    return out, res


def kernel(quaternion, log_scale):
    out, _ = kernel_with_stats(quaternion, log_scale, trace=False)
    return out


# revision 31
# speedup vs baseline: 1.0014x; 1.0014x over previous
"""Trainium2 Bass kernel: per-point 3x3 Gaussian covariance from quaternion + log_scale.

cov = R diag(exp(log_scale)) R^T with R built from the normalized quaternion.

v4: fused-instruction design.
- Host uploads PLANAR fp16 component blocks concatenated per partition row:
  q_cat [P, 4, R] in component order (w, z, y, x), l_cat [P, 3, R].
  Host reassembles the 6 unique cov entries (out_cat [P, 6, R] fp16) into the
  [N,3,3] fp32 output.
- All intermediates fp16 -> DVE tensor_tensor runs in 2x mode.
- Concatenated work tiles + strided/broadcast access patterns fuse groups of
  identical elementwise ops into single wide DVE instructions (e.g. all 3
  columns of M = H diag(sh) in one op), cutting per-instruction fixed cost and
  semaphore traffic.
- ScalarE does the squares (of q and of M) and exps in 4 activations per tile,
  all from one activation table set; 1/(n2/2) via custom-DVE
  reciprocal_approx_fast (fp32).

Math: with half-squares hc = c^2/2 and H = (n2/2) R:
  pa = hw - hz, pb = hx - hy, pc = hw + hz, pd = hx + hy
  h00 = pa + pb, h11 = pa - pb, h22 = pc - pd, n2h = pc + pd
  h01 = xy - wz, h10 = xy + wz, h02 = xz + wy, h20 = xz - wy,
  h12 = yz - wx, h21 = yz + wx
  sh_j = exp(ls_j/2) / n2h;  M = H diag(sh);  cov = M M^T.
"""

import os
import numpy as np

import concourse.bass as bass
import concourse.bacc as bacc
import concourse.mybir as mybir
from concourse.tile import TileContext
from concourse.bass_utils import run_bass_kernel_spmd

AF = mybir.ActivationFunctionType
FP32 = mybir.dt.float32
FP16 = mybir.dt.float16

N_CORES = 8
N_FULL = 4_000_000
P = 128
R = 3920                      # rows per partition per core; 128*3920*8 = 4,014,080 >= N
NPC = P * R                   # points per core (padded)
F = int(os.environ.get("KERNEL_F", "980"))       # points per partition per tile
NT = R // F                                      # tiles per core (uniform)
assert NT * F == R
WORK_BUFS = int(os.environ.get("WORK_BUFS", "1"))
IO_BUFS = int(os.environ.get("IO_BUFS", "2"))
NO_BCAST = os.environ.get("NO_BCAST", "0") == "1"   # fallback: no stride-0 operands
SPLIT_OUT = os.environ.get("SPLIT_OUT", "1") == "1"  # two out-DMAs per tile

SQRT_HALF = 0.7071067811865476

# q_cat component order
QW, QZ, QY, QX = 0, 1, 2, 3

_built = {}


def _build():
    key = (F, WORK_BUFS, IO_BUFS, NO_BCAST, SPLIT_OUT)
    if key in _built:
        return _built[key]

    nc = bacc.Bacc("TRN2", target_bir_lowering=False, debug=False, num_devices=N_CORES)
    # tile-major layouts: each tile's data is contiguous per partition
    q_cat = nc.dram_tensor("q_cat", [P, NT, 4 * F], FP16, kind="ExternalInput")
    l_cat = nc.dram_tensor("l_cat", [P, NT, 3 * F], FP16, kind="ExternalInput")
    o_cat = nc.dram_tensor("o_cat", [P, NT, 6 * F], FP16, kind="ExternalOutput")

    qv, lv, ov = q_cat.ap(), l_cat.ap(), o_cat.ap()

    with TileContext(nc) as tc:
        with (
            tc.tile_pool(name="io", bufs=IO_BUFS) as io,
            tc.tile_pool(name="otp", bufs=2) as ot_pool,
            tc.tile_pool(name="wk", bufs=WORK_BUFS) as wk,
        ):
            for ti in range(NT):
                _tile_body(nc, io, ot_pool, wk, qv, lv, ov, ti, F)

    nc.compile()
    _built[key] = nc
    return nc


def _tile_body(nc, io, ot_pool, wk, qv, lv, ov, ti, f):
    t0 = ti * f

    def wt(tag, units, dt=FP16):
        return wk.tile([P, units * f], dt, tag=tag, name=f"{tag}_{t0}")

    V = nc.vector

    # ---- DMA in (contiguous per partition) -------------------------------
    qt = io.tile([P, 4 * f], FP16, tag="q", name=f"q_{t0}")
    lt = io.tile([P, 3 * f], FP16, tag="l", name=f"l_{t0}")
    nc.sync.dma_start(out=qt, in_=qv[:, ti, :])
    nc.sync.dma_start(out=lt, in_=lv[:, ti, :])
    q3 = qt.rearrange("p (k f) -> p k f", k=4)

    # ---- ScalarE: half-squares (one act), exps (one act) -----------------
    hsq = wt("hsq", 4)
    nc.scalar.activation(hsq, qt, AF.Square, scale=SQRT_HALF)
    hs3 = hsq.rearrange("p (k f) -> p k f", k=4)   # (hw, hz, hy, hx)
    ec = wt("ec", 3)
    nc.scalar.activation(ec, lt, AF.Exp, scale=0.5)

    # ---- pair products first: DVE needs only the q DMA -------------------
    # p layout [pxy pxz pzy pwz pwy pwx]
    pp = wt("pp", 6)
    p6 = pp.rearrange("p (k f) -> p k f", k=6)
    if NO_BCAST:
        for j, qi in enumerate((QZ, QY, QX)):
            V.tensor_mul(p6[:, 3 + j, :], q3[:, QW, :], q3[:, qi, :])
    else:
        wb = q3[:, QW:QW + 1, :].broadcast_to((P, 3, f))
        V.tensor_mul(p6[:, 3:6, :], wb, q3[:, 1:4, :])   # (pwz, pwy, pwx)
    V.tensor_mul(p6[:, 0, :], q3[:, QX, :], q3[:, QY, :])  # pxy
    V.tensor_mul(p6[:, 1, :], q3[:, QX, :], q3[:, QZ, :])  # pxz
    V.tensor_mul(p6[:, 2, :], q3[:, QZ, :], q3[:, QY, :])  # pzy = yz

    # ---- diagonal butterfly ---------------------------------------------
    # spd layout: [pa pb pc pd]
    spd = wt("spd", 4)
    sp3 = spd.rearrange("p (k f) -> p k f", k=4)
    in0 = hs3[:, 0:4:3, :]                # (hw, hx)
    in1 = hs3[:, 1:3, :]                  # (hz, hy)
    V.tensor_sub(sp3[:, 0:2, :], in0, in1)     # (pa, pb)
    V.tensor_add(sp3[:, 2:4, :], in0, in1)     # (pc, pd)
    # hdiag layout: [h00 h11 h22 n2h]
    hd = wt("hd", 4)
    hd3 = hd.rearrange("p (k f) -> p k f", k=4)
    i0 = sp3[:, 0:4:2, :]                 # (pa, pc)
    i1 = sp3[:, 1:4:2, :]                 # (pb, pd)
    V.tensor_sub(hd3[:, 1:3, :], i0, i1)       # (h11, h22)
    V.tensor_add(hd3[:, 0:4:3, :], i0, i1)     # (h00, n2h)

    # ---- reciprocal of n2h ----------------------------------------------
    n2h32 = wt("n2h32", 1, FP32)
    nc.scalar.copy(out=n2h32, in_=hd[:, 3 * f:4 * f])
    inv32 = wt("inv32", 1, FP32)
    V.reciprocal_approx_fast(out=inv32, in_=n2h32)
    inv = wt("inv", 1)
    nc.scalar.copy(out=inv, in_=inv32)

    # ---- off-diagonal H (overlaps the ScalarE inv copy) -----------------
    # ho layout [h01 h20 h12 | h10 h02 h21]
    ho = wt("ho", 6)
    V.tensor_sub(ho[:, 0:3 * f], pp[:, 0:3 * f], pp[:, 3 * f:6 * f])
    V.tensor_add(ho[:, 3 * f:6 * f], pp[:, 0:3 * f], pp[:, 3 * f:6 * f])

    # ---- sh = e * inv (broadcast) ---------------------------------------
    sh = wt("sh", 3)
    if NO_BCAST:
        for j in range(3):
            V.tensor_mul(sh[:, j * f:(j + 1) * f], ec[:, j * f:(j + 1) * f], inv)
    else:
        invb = inv.rearrange("p (k f) -> p k f", k=1).broadcast_to((P, 3, f))
        V.tensor_mul(sh.rearrange("p (k f) -> p k f", k=3),
                     ec.rearrange("p (k f) -> p k f", k=3), invb)

    # ---- M = H diag(sh); M layout j-major: [M00 M10 M20 M01 M11 M21 M02 M12 M22]
    mm = wt("mm", 9)
    m9 = mm.rearrange("p (k f) -> p k f", k=9)
    # diagonal: (h00,h11,h22) * (sh0,sh1,sh2) -> slots 0,4,8
    sh3 = sh.rearrange("p (k f) -> p k f", k=3)
    V.tensor_mul(m9[:, 0:9:4, :], hd3[:, 0:3, :], sh3)
    # off-diagonals, pair-fused: ho=[h01 h20 h12 h10 h02 h21], slot = 3j+i
    ho6 = ho.rearrange("p (k f) -> p k f", k=6)
    # (h01,h12)*(sh1,sh2) -> M(3,7); (h20,h21)*(sh0,sh1) -> M(2,5);
    # (h10,h02)*(sh0,sh2) -> M(1,6)
    V.tensor_mul(m9[:, 3:8:4, :], ho6[:, 0:3:2, :], sh3[:, 1:3, :])
    V.tensor_mul(m9[:, 2:6:3, :], ho6[:, 1:6:4, :], sh3[:, 0:2, :])
    V.tensor_mul(m9[:, 1:7:5, :], ho6[:, 3:5, :], sh3[:, 0:3:2, :])

    # ---- ScalarE: squares of M (one act) --------------------------------
    msq = wt("msq", 9)
    nc.scalar.activation(msq, mm, AF.Square)
    mq9 = msq.rearrange("p (k f) -> p k f", k=9)

    # ---- gram ------------------------------------------------------------
    ot = ot_pool.tile([P, 6 * f], FP16, tag="oc", name=f"oc_{t0}")
    o6 = ot.rearrange("p (k f) -> p k f", k=6)
    # off-diag products: g layout pair-major [(01)j0 j1 j2 | (02)... | (12)...]
    gg = wt("gg", 9)
    g9 = gg.rearrange("p (k f) -> p k f", k=9)
    if NO_BCAST:
        for pi, (i, k) in enumerate(((0, 1), (0, 2), (1, 2))):
            V.tensor_mul(g9[:, 3 * pi:3 * (pi + 1), :],
                         m9[:, i:9:3, :], m9[:, k:9:3, :])
    else:
        # pairs (0,1) and (0,2) share row0: one 6f op with a broadcast row0
        mij = mm.rearrange("p (j i f) -> p i j f", j=3, i=3)
        row0b = mij[:, 0:1, :, :].broadcast_to((P, 2, 3, f))
        g01_02 = gg[:, 0:6 * f].rearrange("p (q j f) -> p q j f", q=2, j=3)
        V.tensor_mul(g01_02, row0b, mij[:, 1:3, :, :])
        V.tensor_mul(g9[:, 6:9, :], m9[:, 1:9:3, :], m9[:, 2:9:3, :])
    s3 = wt("s3", 3)
    V.tensor_add(s3.rearrange("p (k f) -> p k f", k=3),
                 g9[:, 0:9:3, :], g9[:, 1:9:3, :])
    V.tensor_add(o6[:, 0:3, :],
                 s3.rearrange("p (k f) -> p k f", k=3), g9[:, 2:9:3, :])
    if SPLIT_OUT:
        nc.sync.dma_start(out=ov[:, ti, 0:3 * f], in_=ot[:, 0:3 * f])
    # diag: cov_ii = sum_j msq[3j+i]
    sd = wt("sd", 3)
    V.tensor_add(sd, msq[:, 0:3 * f], msq[:, 3 * f:6 * f])
    V.tensor_add(o6[:, 3:6, :],
                 sd.rearrange("p (k f) -> p k f", k=3), mq9[:, 6:9, :])
    if SPLIT_OUT:
        nc.sync.dma_start(out=ov[:, ti, 3 * f:6 * f], in_=ot[:, 3 * f:6 * f])
    else:
        nc.sync.dma_start(out=ov[:, ti, :], in_=ot)


def _pad_and_shard(quaternion, log_scale):
    n = quaternion.shape[0]
    ntot = N_CORES * NPC
    q16 = np.empty((ntot, 4), np.float16)
    l16 = np.empty((ntot, 3), np.float16)
    q16[:n] = quaternion.astype(np.float16)
    l16[:n] = log_scale.astype(np.float16)
    q16[n:] = np.array([1, 0, 0, 0], np.float16)
    l16[n:] = 0
    in_maps = []
    comp_order = (0, 3, 2, 1)   # (w, z, y, x) columns of quaternion
    for i in range(N_CORES):
        sl = slice(i * NPC, (i + 1) * NPC)
        qc = np.empty((P, NT, 4, F), np.float16)
        for k, ci in enumerate(comp_order):
            qc[:, :, k, :] = q16[sl, ci].reshape(P, NT, F)
        lc = np.empty((P, NT, 3, F), np.float16)
        for ci in range(3):
            lc[:, :, ci, :] = l16[sl, ci].reshape(P, NT, F)
        in_maps.append({"q_cat": qc.reshape(P, NT, 4 * F),
                        "l_cat": lc.reshape(P, NT, 3 * F)})
    return in_maps


def kernel_with_stats(quaternion, log_scale, trace=False):
    quaternion = np.asarray(quaternion, dtype=np.float32)
    log_scale = np.asarray(log_scale, dtype=np.float32)
    n = quaternion.shape[0]
    nc = _build()
    in_maps = _pad_and_shard(quaternion, log_scale)
    res = run_bass_kernel_spmd(nc, in_maps, core_ids=list(range(N_CORES)), trace=trace)
    out = np.empty((n, 3, 3), np.float32)
    # o_cat slots: [c01 c02 c12 | c00 c11 c22]
    slots = {0: [(0, 1), (1, 0)], 1: [(0, 2), (2, 0)], 2: [(1, 2), (2, 1)],
             3: [(0, 0)], 4: [(1, 1)], 5: [(2, 2)]}
    for s, ps in slots.items():
        full = np.concatenate(
            [r["o_cat"].reshape(P, NT, 6, F)[:, :, s, :].reshape(-1)
             for r in res.results])[:n]
        full = full.astype(np.float32)
        for (i, k) in ps:
            out[:, i, k] = full
    return out, res


def kernel(quaternion, log_scale):
    out, _ = kernel_with_stats(quaternion, log_scale, trace=False)
    return out
